# revision 6
# baseline (speedup 1.0000x reference)
"""Bilinear sampling (dense_image_warp) Trainium2 kernel — quad-gather v3.

Design notes (what makes this fast):
- Host precomputes a bf16 "quad" image layout: quad[r, x] = the 2x2 bilinear
  neighborhood [(r,x),(r,x+1),(r+1,x),(r+1,x+1)] x 32ch as one contiguous
  256B block.  One dma_gather descriptor per output pixel, zero overfetch.
- dma_gather descriptor generation runs on a single Q7 core pair selected by
  queue_num (~8ns/descriptor) and is THE bottleneck; gathers are issued
  round-robin across all 4 SWDGE queues so ring drains never stall the next
  gather's descriptor writes.
- DVE ops that can enter 2-port perf mode (tensor_scalar / cast / copy)
  fully block against SWDGE descriptor generation, so the kernel avoids them
  entirely during the gather stream: floor() is computed with the
  magic-number trick (bilinear continuity makes round-vs-floor differences
  harmless), clips/casts are tensor_tensor against constant tiles, and dtype
  converts ride output-dtype conversion of tensor_tensor.
- Indices are band-relative (128-row bands per 64 output rows; |256*flow| <
  18 guaranteed by clamping) so they fit int16.  The wrapped+replicated
  int16 index layout dma_gather wants is produced with 8 PE matmuls against
  0/1 selection matrices (exact in f32).
- Quad elements are packed [ch][k] so the blend is ONE contiguous in-place
  bf16 multiply against interleaved weights + ONE tensor_reduce over the
  innermost k=4 axis (f32 accumulation): ~2 DVE ops per gather call.
"""

import os
import sys

import numpy as np

for _p in ("/opt/trn_rl_repo", "/root/.axon_site/_ro/trn_rl_repo"):
    if os.path.isdir(_p) and _p not in sys.path:
        sys.path.append(_p)

NCORES = 8
B, H, W, C = 32, 256, 256, 32
NS = B // NCORES              # samples per core
NPIX = H * W                  # pixels per sample
NCOLS = NPIX // 128           # 512 G-layout columns per sample
NPAIR = 16                    # gather calls per sample (16 output rows each)
PAIRC = NCOLS // NPAIR        # 32 G-columns per gather call
PAIRPX = PAIRC * 128          # 4096 pixels (= descriptors) per gather call
QELEM = 128                   # bf16 elems per quad (4 px * 32 ch) = 256B
BAND = 48                     # source band height (rows) per gather call
MAGIC = 12582912.0            # 1.5 * 2^23: float add rounds to integer
# band base row per gather call p (output rows 16p..16p+15 read source rows
# [16p-14, 16p+29] clipped; BASES[p]..BASES[p]+47 covers that, margin 2)
# (4096 descriptors per call = 257 of the ring's 1024 16-engine rows, so
# three calls fit per queue ring and generation never stalls on drains.)
BASES = [min(max(16 * p - 16, 0), H - BAND) for p in range(NPAIR)]

_CACHE = {}


def _build_module():
    import concourse.bacc as bacc
    import concourse.mybir as mybir
    import concourse.tile as tile
    from concourse import library_config

    f32 = mybir.dt.float32
    bf16 = mybir.dt.bfloat16
    i16 = mybir.dt.int16
    Alu = mybir.AluOpType
    Act = mybir.ActivationFunctionType

    nc = bacc.Bacc(
        "TRN2", target_bir_lowering=False, debug=False, num_swdge_queues=4
    )

    quads = nc.dram_tensor("quads", [NS, NPIX * QELEM], bf16, kind="ExternalInput")
    flowg = nc.dram_tensor("flowg", [NS, 2, 128, NCOLS], f32, kind="ExternalInput")
    # consts[:, 0:512]=iG+MAGIC  [:,512:1024]=jG+MAGIC
    # [:,1024:1536]=iG  [:,1536:2048]=jG  [:,2048:2560]=256*base(c)
    consts = nc.dram_tensor("consts", [128, 5 * NCOLS], f32, kind="ExternalInput")
    sel = nc.dram_tensor("sel", [128, 1024], f32, kind="ExternalInput")
    out = nc.dram_tensor("out", [NS, 128, NCOLS, C], f32, kind="ExternalOutput")

    def free_view(ap, offset_elems, dims):
        """View of `ap` keeping its partition dim, replacing free dims."""
        v = ap.copy()
        part = v.ap.to_list()[0]
        v.ap.clear()
        v.ap.extend([part] + [list(d) for d in dims])
        v.offset = v.offset + offset_elems
        return v

    with nc.Block() as _blk:
        @_blk.gpsimd
        def _(g):
            g.load_library(library_config.mlp)

    with tile.TileContext(nc) as tc:
        with (
            tc.tile_pool(name="consts", bufs=1) as cpool,
            tc.tile_pool(name="flow", bufs=2) as fpool,
            tc.tile_pool(name="wts", bufs=2) as wpool,
            tc.tile_pool(name="wtmp", bufs=1) as xpool,
            tc.tile_pool(name="idx", bufs=2) as ipool,
            tc.tile_pool(name="psum", bufs=4, space="PSUM") as ppool,
            tc.tile_pool(name="gat", bufs=10) as gpool,
            tc.tile_pool(name="outp", bufs=2) as opool,
        ):
            V, A = nc.vector, nc.scalar

            ct = cpool.tile([128, 5 * NCOLS], f32)
            nc.sync.dma_start(ct[:], consts[:])
            selt = cpool.tile([128, 1024], f32)
            nc.sync.dma_start(selt[:], sel[:])
            iGm = ct[:, 0:NCOLS]
            jGm = ct[:, NCOLS : 2 * NCOLS]
            iG = ct[:, 2 * NCOLS : 3 * NCOLS]
            jG = ct[:, 3 * NCOLS : 4 * NCOLS]
            baseT = ct[:, 4 * NCOLS : 5 * NCOLS]
            # constant tiles (memset before any gather issues)
            magicT = cpool.tile([128, NCOLS], f32)
            V.memset(magicT[:], MAGIC)
            zeroT = cpool.tile([128, NCOLS], f32)
            V.memset(zeroT[:], 0.0)
            oneT = cpool.tile([128, NCOLS], f32)
            V.memset(oneT[:], 1.0)
            c254T = cpool.tile([128, NCOLS], f32)
            V.memset(c254T[:], 254.0)
            cMaxT = cpool.tile([128, NCOLS], f32)
            V.memset(cMaxT[:], 16383.0)

            gather_no = 0
            per_sample = []
            # one shared register for num_idxs: a fresh MOVE per gather
            # creates a WAR hazard that serializes gather launches
            nidx_reg = nc.gpsimd.to_reg(PAIRPX)

            def emit_idx_chain(s):
                ft = fpool.tile([128, 2 * NCOLS], f32, tag="ft", name="ft")
                nc.sync.dma_start(ft[:, 0:NCOLS], flowg[s, 0])
                nc.sync.dma_start(ft[:, NCOLS : 2 * NCOLS], flowg[s, 1])
                dy = ft[:, 0:NCOLS]
                dx = ft[:, NCOLS : 2 * NCOLS]

                def xt(tag):
                    return xpool.tile([128, NCOLS], f32, tag=tag, name=tag)

                qym, qxm = xt("qym"), xt("qxm")
                qy, qx = xt("qy"), xt("qx")
                fy, fx = xt("fy"), xt("fx")
                ay, ax = xt("ay"), xt("ax")
                ayc, axc = qym, qxm  # qym/qxm are dead once fy/fx exist
                idxf = xt("idxf")
                msk = idxf  # idxf is only written after the masks are consumed

                # qym = -256*dy + (iG + MAGIC): the float add rounds to an
                # integer (ulp=1 near 1.5*2^23), so qym - MAGIC = round(qy).
                # floor = round - (qy - round < 0), all 1-port tensor_tensor.
                V.scalar_tensor_tensor(
                    out=qym[:], in0=dy, scalar=-256.0, in1=iGm, op0=Alu.mult, op1=Alu.add
                )
                V.scalar_tensor_tensor(
                    out=qxm[:], in0=dx, scalar=-256.0, in1=jGm, op0=Alu.mult, op1=Alu.add
                )
                V.scalar_tensor_tensor(
                    out=qy[:], in0=dy, scalar=-256.0, in1=iG, op0=Alu.mult, op1=Alu.add
                )
                V.scalar_tensor_tensor(
                    out=qx[:], in0=dx, scalar=-256.0, in1=jG, op0=Alu.mult, op1=Alu.add
                )
                # fy = clip(floor(qy), 0, 254)
                V.tensor_tensor(out=fy[:], in0=qym[:], in1=magicT[:], op=Alu.subtract)
                V.tensor_tensor(out=msk[:], in0=qy[:], in1=fy[:], op=Alu.is_lt)
                V.tensor_tensor(out=fy[:], in0=fy[:], in1=msk[:], op=Alu.subtract)
                V.tensor_tensor(out=fy[:], in0=fy[:], in1=zeroT[:], op=Alu.max)
                V.tensor_tensor(out=fy[:], in0=fy[:], in1=c254T[:], op=Alu.min)
                # fx = clip(floor(qx), 0, 254)
                V.tensor_tensor(out=fx[:], in0=qxm[:], in1=magicT[:], op=Alu.subtract)
                V.tensor_tensor(out=msk[:], in0=qx[:], in1=fx[:], op=Alu.is_lt)
                V.tensor_tensor(out=fx[:], in0=fx[:], in1=msk[:], op=Alu.subtract)
                V.tensor_tensor(out=fx[:], in0=fx[:], in1=zeroT[:], op=Alu.max)
                V.tensor_tensor(out=fx[:], in0=fx[:], in1=c254T[:], op=Alu.min)
                # ---- index fold (everything gathers need) ----
                idxw = ipool.tile([128, NPIX // 16], i16, tag="idxw", name="idxw")
                V.scalar_tensor_tensor(
                    out=idxf[:], in0=fy[:], scalar=256.0, in1=fx[:],
                    op0=Alu.mult, op1=Alu.add,
                )
                V.tensor_tensor(out=idxf[:], in0=idxf[:], in1=baseT, op=Alu.subtract)
                V.tensor_tensor(out=idxf[:], in0=idxf[:], in1=zeroT[:], op=Alu.max)
                V.tensor_tensor(out=idxf[:], in0=idxf[:], in1=cMaxT[:], op=Alu.min)
                for g in range(8):
                    ps = ppool.tile([128, NCOLS], f32, tag="ps", name="ps")
                    nc.tensor.matmul(
                        ps[:], lhsT=selt[:, g * 128 : (g + 1) * 128], rhs=idxf[:],
                        start=True, stop=True,
                    )
                    src = free_view(ps[:], 0, [[PAIRC, NPAIR], [1, PAIRC]])
                    dst = free_view(idxw[:], g, [[PAIRPX // 16, NPAIR], [8, PAIRC]])
                    # ACT engine (idle) does the psum->int16 fold copy so it
                    # never queues behind DVE blends
                    A.activation(out=dst, in_=src, func=Act.Copy, bias=0.0, scale=1.0)
                return (idxw, qy, qx, fy, fx, ay, ax, ayc, axc)

            def emit_w_chain(s, saved):
                idxw, qy, qx, fy, fx, ay, ax, ayc, axc = saved
                # ay = clip(qy - fy, 0, 1); ax likewise
                V.tensor_tensor(out=ay[:], in0=qy[:], in1=fy[:], op=Alu.subtract)
                V.tensor_tensor(out=ay[:], in0=ay[:], in1=zeroT[:], op=Alu.max)
                V.tensor_tensor(out=ay[:], in0=ay[:], in1=oneT[:], op=Alu.min)
                V.tensor_tensor(out=ax[:], in0=qx[:], in1=fx[:], op=Alu.subtract)
                V.tensor_tensor(out=ax[:], in0=ax[:], in1=zeroT[:], op=Alu.max)
                V.tensor_tensor(out=ax[:], in0=ax[:], in1=oneT[:], op=Alu.min)
                # complements
                V.tensor_tensor(out=ayc[:], in0=oneT[:], in1=ay[:], op=Alu.subtract)
                V.tensor_tensor(out=axc[:], in0=oneT[:], in1=ax[:], op=Alu.subtract)
                # bilinear weights, interleaved [c*4 + k] bf16 (k = tl,tr,bl,br)
                wq = wpool.tile([128, NCOLS * 4], bf16, tag="wq", name="wq")
                for k, (wy, wx) in enumerate(((ayc, axc), (ayc, ax), (ay, axc), (ay, ax))):
                    wk = free_view(wq[:], k, [[4, NCOLS]])
                    V.tensor_tensor(out=wk, in0=wy[:], in1=wx[:], op=Alu.mult)
                per_sample.append((idxw, wq))

            # ---- per gather call: gather + blend ----
            saved0 = emit_idx_chain(0)
            emit_w_chain(0, saved0)
            saved_next = [None]
            for s in range(NS):
                idxw, wq = per_sample[s]
                for p in range(NPAIR):
                    if p == 2 and s + 1 < NS:
                        saved_next[0] = emit_idx_chain(s + 1)
                    if p == 9 and s + 1 < NS:
                        emit_w_chain(s + 1, saved_next[0])
                    gt_ = gpool.tile([128, PAIRC * QELEM], bf16, tag="g", name="g")
                    g3 = gt_[:].rearrange("p (a b) -> p a b", a=PAIRC)
                    src = quads[:].copy()
                    src.ap.clear()
                    src.ap.extend([[QELEM, BAND * W], [1, QELEM]])
                    src.offset = (s * NPIX + BASES[p] * W) * QELEM
                    idx_ap = idxw[:, p * (PAIRPX // 16) : (p + 1) * (PAIRPX // 16)]
                    nc.gpsimd.dma_gather(
                        g3, src, idx_ap,
                        num_idxs=PAIRPX, num_idxs_reg=nidx_reg,
                        elem_size=QELEM, elem_step=QELEM, single_packet=False,
                        queue_num=gather_no % 4,
                    )
                    gather_no += 1
                    ot = opool.tile([128, PAIRC * C], f32, tag="ot", name="ot")
                    ot3 = ot[:].rearrange("p (a b) -> p a b", a=PAIRC)

                    # gt elements are [ch][k]: multiply in place by the
                    # interleaved weights (broadcast over ch), then reduce
                    # the innermost k=4 axis with f32 accumulation.
                    g4 = free_view(gt_[:], 0, [[QELEM, PAIRC], [4, C], [1, 4]])
                    w4 = free_view(wq[:], p * PAIRC * 4, [[4, PAIRC], [0, C], [1, 4]])
                    V.tensor_tensor(out=g4, in0=g4, in1=w4, op=Alu.mult)
                    V.tensor_reduce(
                        out=ot3, in_=g4, axis=mybir.AxisListType.X, op=Alu.add
                    )

                    nc.sync.dma_start(
                        out[s, :, p * PAIRC : (p + 1) * PAIRC, :], ot3
                    )

    nc.compile()
    return nc


def _host_constants():
    p = np.arange(128)[:, None]
    c = np.arange(NCOLS)[None, :]
    m = c * 128 + p
    iG = (m // W).astype(np.float32)
    jG = (m % W).astype(np.float32)
    baseT = np.broadcast_to(
        np.repeat(np.float32([256.0 * b for b in BASES]), PAIRC)[None, :], (128, NCOLS)
    )
    consts = np.concatenate(
        [iG + np.float32(MAGIC), jG + np.float32(MAGIC), iG, jG, baseT],
        axis=1,
    ).astype(np.float32)
    # sel: sel[p, g*128+q] = 1 if p == g*16 + q%16
    sel = np.zeros((128, 1024), np.float32)
    for g in range(8):
        for q in range(128):
            sel[g * 16 + (q % 16), g * 128 + q] = 1.0
    return consts, sel


def _prep_core_inputs(image, flow, core):
    import ml_dtypes

    sl = slice(core * NS, (core + 1) * NS)
    img = np.asarray(image[sl], dtype=np.float32).astype(ml_dtypes.bfloat16)
    # element layout [ch][k]: innermost k = (tl, tr, bl, br) for tensor_reduce
    q = np.zeros((NS, H, W, C, 4), ml_dtypes.bfloat16)
    q[:, :, :, :, 0] = img
    q[:, :, :-1, :, 1] = img[:, :, 1:]
    q[:, :-1, :, :, 2] = img[:, 1:]
    q[:, :-1, :-1, :, 3] = img[:, 1:, 1:]
    quads = q.reshape(NS, NPIX * QELEM)
    # flowg[s, ch, p, c] = flow[s, pixel c*128+p, ch]
    fl = np.ascontiguousarray(flow[sl], dtype=np.float32).reshape(NS, NCOLS, 128, 2)
    flowg = np.ascontiguousarray(fl.transpose(0, 3, 2, 1))
    return quads, flowg


def kernel(image, flow):
    from concourse import bass_utils

    image = np.asarray(image, dtype=np.float32)
    flow = np.asarray(flow, dtype=np.float32)

    if "nc" not in _CACHE:
        _CACHE["nc"] = _build_module()
        _CACHE["consts"], _CACHE["sel"] = _host_constants()
    nc = _CACHE["nc"]
    consts, sel = _CACHE["consts"], _CACHE["sel"]

    in_maps = []
    for core in range(NCORES):
        quads, flowg = _prep_core_inputs(image, flow, core)
        in_maps.append({"quads": quads, "flowg": flowg, "consts": consts, "sel": sel})

    trace = os.environ.get("BILIN_TRACE", "") == "1"
    kw = {}
    if trace:
        kw["trace"] = True
        td = os.environ.get("BILIN_TRACE_DIR")
        if td:
            os.makedirs(td, exist_ok=True)
            kw["tmpdir"] = td
    res = bass_utils.run_bass_kernel_spmd(
        nc, in_maps, core_ids=list(range(NCORES)), **kw
    )
    _CACHE["last_res"] = res
    if trace and res.exec_time_ns is not None:
        print(f"[trace] exec_time_ns: {res.exec_time_ns}", file=sys.stderr)

    outs = []
    for r in res.results:
        o = r["out"]  # [NS, 128, 512, 32]; pixel m = c*128+p at [s, p, c, :]
        outs.append(o.transpose(0, 2, 1, 3).reshape(NS, H, W, C))
    return np.concatenate(outs, axis=0)
